# revision 2
# baseline (speedup 1.0000x reference)
"""Trainium2 Bass kernel for nn_Net_76330158785143 (dense_cnn).

Pipeline per sample: per-sample 11x11 autocorrelation of channel 2 ->
conv5x5(1->32) relu -> maxpool2 -> conv5x5(32->64) relu -> maxpool2 ->
conv3x3(64->10) relu -> GAP -> log_softmax.

Sharding: pure data parallel, batch 8192 -> 1024 per core across 8 cores.
"""

import sys

sys.path.insert(0, "/opt/trn_rl_repo")

import numpy as np

import concourse.bacc as bacc
import concourse.mybir as mybir
from concourse.ap import AP
from concourse.tile import TileContext
from concourse.bass_utils import run_bass_kernel_spmd

F32 = mybir.dt.float32
BF16 = mybir.dt.bfloat16
ALU = mybir.AluOpType
ACTF = mybir.ActivationFunctionType
AXIS = mybir.AxisListType
POOLF = mybir.PoolFunctionType

DEBUG_TAPS = False
N_CORES = 8
B_FULL = 8192
B_CORE = B_FULL // N_CORES


def _build(nc, b_core):
    """Emit the full per-core program for b_core samples (multiple of 128)."""
    n_bt = b_core // 128  # 128-sample tiles

    x_d = nc.dram_tensor("x", [b_core, 3, 28, 28], F32, kind="ExternalInput")
    identp_d = nc.dram_tensor("identp", [128, 128], BF16, kind="ExternalInput")
    ident10p_d = nc.dram_tensor("ident10p", [16, 16], F32, kind="ExternalInput")
    w1p_d = nc.dram_tensor("w1p", [32, 32], BF16, kind="ExternalInput")
    w2p_d = nc.dram_tensor("w2p", [128, 448], BF16, kind="ExternalInput")
    w3p_d = nc.dram_tensor("w3p", [128, 56], BF16, kind="ExternalInput")
    b1p_d = nc.dram_tensor("b1p", [128, 1], F32, kind="ExternalInput")
    b2p_d = nc.dram_tensor("b2p", [64, 1], F32, kind="ExternalInput")
    b3p_d = nc.dram_tensor("b3p", [16, 1], F32, kind="ExternalInput")
    out_d = nc.dram_tensor("out", [b_core, 10], F32, kind="ExternalOutput")
    dbg = {}
    if DEBUG_TAPS:
        for nm, shp, dt in [
            ("dbg_corr", [128, 924], BF16), ("dbg_s25", [32, 21504], BF16),
            ("dbg_a1", [128, 288], BF16), ("dbg_py", [128, 72], BF16),
            ("dbg_dup4", [128, 4608], BF16), ("dbg_t3", [64, 512], F32),
            ("dbg_o2", [64, 128], BF16), ("dbg_lgr", [16, 128], F32),
            ("dbg_logitsb", [16, 128], F32),
        ]:
            dbg[nm] = nc.dram_tensor(nm, shp, dt, kind="ExternalOutput")

    with TileContext(nc) as tc:
        cpool_cm = tc.tile_pool(name="const", bufs=1)
        cpool = cpool_cm.__enter__()

        def _load_const(name, dram, shape, dtype):
            t = cpool.tile(shape, dtype, name=name + "_sb")
            f = int(np.prod(shape[1:]))
            nc.sync.dma_start(
                out=AP(t.tensor, 0, [[f, shape[0]], [1, f]]),
                in_=AP(dram, 0, [[f, shape[0]], [1, f]]),
            )
            return t

        # host-prearranged constants (see _prep_inputs)
        ident = _load_const("ident", identp_d, [128, 128], BF16)
        ident10 = _load_const("ident10", ident10p_d, [16, 16], F32)
        w1_sb = _load_const("w1p", w1p_d, [32, 32], BF16)
        w2_sb = _load_const("w2p", w2p_d, [128, 448], BF16)
        w3_sb = _load_const("w3p", w3p_d, [128, 56], BF16)
        b1_sb = _load_const("b1p", b1p_d, [128, 1], F32)
        b2_sb = _load_const("b2p", b2p_d, [64, 1], F32)
        b3_sb = _load_const("b3p", b3p_d, [16, 1], F32)

        with (
            tc.tile_pool(name="img", bufs=2) as imgpool,
            tc.tile_pool(name="tmpl", bufs=2) as tmplpool,
            tc.tile_pool(name="diag", bufs=10) as diagpool,
            tc.tile_pool(name="corr", bufs=2) as corrpool,
            tc.tile_pool(name="s25", bufs=2) as s25pool,
            tc.tile_pool(name="a1", bufs=3) as a1pool,
            tc.tile_pool(name="pool1", bufs=3) as p1pool,
            tc.tile_pool(name="dup4", bufs=2) as dup4pool,
            tc.tile_pool(name="t2", bufs=6) as t2pool,
            tc.tile_pool(name="o2", bufs=3) as o2pool,
            tc.tile_pool(name="l3", bufs=2) as l3pool,
            tc.tile_pool(name="sm", bufs=4) as smpool,
            tc.tile_pool(name="lgb", bufs=2) as lgbpool,
            tc.tile_pool(name="dscr", bufs=2, space="DRAM") as dscrpool,
            tc.tile_pool(name="pcorr", bufs=1, space="PSUM") as pcorr,
            tc.tile_pool(name="pc1", bufs=2, space="PSUM") as pc1,
            tc.tile_pool(name="pc2", bufs=2, space="PSUM") as pc2,
        ):
            pools = dict(
                imgpool=imgpool, tmplpool=tmplpool, diagpool=diagpool,
                corrpool=corrpool, s25pool=s25pool, a1pool=a1pool,
                p1pool=p1pool, dup4pool=dup4pool,
                t2pool=t2pool, o2pool=o2pool, l3pool=l3pool, smpool=smpool,
                lgbpool=lgbpool, dscrpool=dscrpool, pcorr=pcorr, pc1=pc1, pc2=pc2,
            )
            consts = dict(
                ident=ident, ident10=ident10, w1_sb=w1_sb, w2_sb=w2_sb,
                w3_sb=w3_sb, b1_sb=b1_sb, b2_sb=b2_sb, b3_sb=b3_sb,
            )
            for bt in range(n_bt):
                _do_btile(nc, bt, b_core, x_d, out_d, dbg, pools, consts)

        cpool_cm.__exit__(None, None, None)
    return nc


_CACHE = {}


def _get_nc(b_core):
    if b_core not in _CACHE:
        nc = bacc.Bacc("TRN2", target_bir_lowering=False, debug=False, num_devices=N_CORES)
        _build(nc, b_core)
        nc.compile()
        _CACHE[b_core] = nc
    return _CACHE[b_core]


def _prep_inputs(inputs):
    import ml_dtypes

    bf16 = ml_dtypes.bfloat16
    w1 = np.asarray(inputs["w1"], dtype=np.float32).reshape(32, 25)
    w2 = np.asarray(inputs["w2"], dtype=np.float32).reshape(64, 32, 25)
    w3 = np.asarray(inputs["w3"], dtype=np.float32).reshape(10, 64, 9)
    b1 = np.asarray(inputs["b1"], dtype=np.float32)
    b2 = np.asarray(inputs["b2"], dtype=np.float32)
    b3 = np.asarray(inputs["b3"], dtype=np.float32)

    w1p = np.ascontiguousarray(w1.T).astype(bf16)  # [25t, 32co] -> pad [32, 32]
    w1p = np.pad(w1p, ((0, 7), (0, 0)))
    w2p = np.zeros((128, 448), dtype=bf16)
    for t in range(25):
        r, slot = t % 4, t // 4
        # [ci, co] block at partitions 32r.., free slot*64..
        w2p[32 * r : 32 * r + 32, slot * 64 : slot * 64 + 64] = w2[:, :, t].T.astype(
            bf16
        )
    w3p = np.zeros((128, 56), dtype=bf16)
    for t in range(9):
        r, slot = t % 2, t // 2
        w3p[64 * r : 64 * r + 64, slot * 10 : slot * 10 + 10] = w3[:, :, t].T.astype(
            bf16
        )
    b1p = np.tile(b1, 4).reshape(128, 1)
    b2p = b2.reshape(64, 1)
    b3p = np.pad(b3, (0, 6)).reshape(16, 1)
    identp = np.eye(128, dtype=bf16)
    ident10p = np.eye(16, dtype=np.float32)
    return dict(
        identp=identp,
        ident10p=ident10p,
        w1p=w1p,
        w2p=w2p,
        w3p=w3p,
        b1p=b1p,
        b2p=b2p,
        b3p=b3p,
    )


def _run(inputs, b_core=B_CORE, trace=False):
    x = np.ascontiguousarray(np.asarray(inputs["x"], dtype=np.float32))
    consts = _prep_inputs(inputs)
    nc = _get_nc(b_core)
    in_maps = [
        {"x": x[i * b_core : (i + 1) * b_core], **consts} for i in range(N_CORES)
    ]
    res = run_bass_kernel_spmd(nc, in_maps, core_ids=list(range(N_CORES)), trace=trace)
    out = np.concatenate([res.results[i]["out"] for i in range(N_CORES)], axis=0)
    return out.astype(np.float32), res


def kernel(**inputs) -> np.ndarray:
    out, _ = _run(inputs)
    return out


def _do_btile(nc, bt, b_core, x_d, out_d, dbg, P, C):
    dscrpool = P["dscrpool"]
    imgpool = P["imgpool"]; tmplpool = P["tmplpool"]; diagpool = P["diagpool"]
    corrpool = P["corrpool"]; s25pool = P["s25pool"]; a1pool = P["a1pool"]
    p1pool = P["p1pool"]; dup4pool = P["dup4pool"]
    t2pool = P["t2pool"]; o2pool = P["o2pool"]; l3pool = P["l3pool"]
    smpool = P["smpool"]; lgbpool = P["lgbpool"]; pcorr = P["pcorr"]
    pc1 = P["pc1"]; pc2 = P["pc2"]
    ident = C["ident"]; ident10 = C["ident10"]; w1_sb = C["w1_sb"]
    w2_sb = C["w2_sb"]; w3_sb = C["w3_sb"]; b1_sb = C["b1_sb"]
    b2_sb = C["b2_sb"]; b3_sb = C["b3_sb"]

    # ---- load channel 2 into zero-padded 38x38, cast bf16 ----
    img = imgpool.tile([128, 38 * 38], BF16)
    nc.gpsimd.memset(img[:, :], 0.0)
    nc.gpsimd.dma_start(
        out=AP(img.tensor, 5 * 38 + 5, [[1444, 128], [38, 28], [1, 28]]),
        in_=AP(
            x_d,
            bt * 128 * 2352 + 2 * 784,
            [[2352, 128], [1, 784]],
        ),
    )
    # template = center 11x11 crop (rows/cols 8..18 of 28x28 content)
    tmpl = tmplpool.tile([128, 128], F32)
    nc.vector.tensor_copy(
        out=AP(tmpl.tensor, 0, [[128, 128], [1, 121]]),
        in_=AP(img.tensor, 13 * 38 + 13, [[1444, 128], [38, 11], [1, 11]]),
    )

    # ---- correlation: 121 accumulating diag matmuls ----
    ps_a = pcorr.tile([128, 392], F32, tag="corr_a")
    ps_b = pcorr.tile([128, 392], F32, tag="corr_b")
    for t in range(121):
        u, v = t // 11, t % 11
        dg = diagpool.tile([128, 128], BF16)
        nc.vector.tensor_scalar_mul(dg[:, :], ident[:, :], tmpl[:, t : t + 1])
        nc.tensor.matmul(
            ps_a[:, :],
            dg[:, :],
            AP(img.tensor, u * 38 + v, [[1444, 128], [38, 14], [1, 28]]),
            start=(t == 0),
            stop=(t == 120),
        )
        nc.tensor.matmul(
            ps_b[:, :],
            dg[:, :],
            AP(
                img.tensor,
                (u + 14) * 38 + v,
                [[1444, 128], [38, 14], [1, 28]],
            ),
            start=(t == 0),
            stop=(t == 120),
        )
    # corr in bf16, flat 784 + zero tail to 924 (shift window slack)
    corr = corrpool.tile([128, 924], BF16)
    nc.vector.tensor_copy(out=corr[:, 0:392], in_=ps_a[:, :])
    nc.vector.tensor_copy(out=corr[:, 392:784], in_=ps_b[:, :])
    nc.gpsimd.memset(corr[:, 784:924], 0.0)
    corr_d = dscrpool.tile([128, 924], BF16, tag="corr_d")
    nc.sync.dma_start(
        out=AP(corr_d.tensor, 0, [[924, 128], [1, 924]]),
        in_=corr[:, :],
    )
    if DEBUG_TAPS and bt == 0:
        nc.sync.dma_start(out=AP(dbg["dbg_corr"], 0, [[924, 128], [1, 924]]), in_=corr[:, :])

    logitsb = lgbpool.tile([16, 128], F32)

    for sub in range(4):  # 32-sample subchunks
        # ---- shift-replicate corr into 25 tap partitions ----
        # s25[p=(dy,dx), s*672 + j] = corr[s, dy*28+dx + j]
        out1p_d = dscrpool.tile([32, 32 * 144], BF16, tag="out1p_d")
        s25 = s25pool.tile([32, 32 * 672], BF16)
        _dbg1 = DEBUG_TAPS and bt == 0 and sub == 0
        for dy in range(5):
            nc.sync.dma_start(
                out=AP(s25.tensor, dy * 5 * 21504, [[21504, 5], [1, 21504]]),
                in_=AP(
                    corr_d.tensor,
                    sub * 32 * 924 + dy * 28,
                    [[1, 5], [924, 32], [1, 672]],
                ),
            )
        if _dbg1:
            nc.sync.dma_start(out=AP(dbg["dbg_s25"], 0, [[21504, 25], [1, 21504]]), in_=AP(s25.tensor, 0, [[21504, 25], [1, 21504]]))
        # ---- conv1: rounds of (4 samples x half-image), 4 col groups
        for q in range(8):
            for h in range(2):
                ps1 = pc1.tile([128, 288], F32, tag="ps1")
                for c in range(4):
                    s_loc = q * 4 + c
                    rhs = AP(
                        s25.tensor,
                        s_loc * 672 + h * 336,
                        [[21504, 25], [28, 12], [1, 24]],
                    )
                    nc.tensor.matmul(
                        ps1[32 * c : 32 * c + 32, :],
                        w1_sb[0:25, :],
                        rhs,
                        start=True,
                        stop=True,
                        tile_position=(0, 32 * c),
                    )
                # bias+relu+cast on ACT: a1 = relu(ps1 + b1)
                a1 = a1pool.tile([128, 288], BF16)
                nc.scalar.activation(
                    a1[:, :], ps1[:, :], ACTF.Relu, bias=b1_sb[:, 0:1]
                )
                if _dbg1 and q == 0 and h == 0:
                    nc.sync.dma_start(out=AP(dbg["dbg_a1"], 0, [[288, 128], [1, 288]]), in_=a1[:, :])
                # maxpool 2x2 (x then y)
                px = p1pool.tile([128, 144], BF16, tag="px")
                nc.vector.tensor_max(
                    px[:, :],
                    AP(a1.tensor, 0, [[288, 128], [24, 12], [2, 12]]),
                    AP(a1.tensor, 1, [[288, 128], [24, 12], [2, 12]]),
                )
                py = p1pool.tile([128, 72], BF16, tag="py")
                nc.vector.tensor_max(
                    py[:, :],
                    AP(px.tensor, 0, [[144, 128], [24, 6], [1, 12]]),
                    AP(px.tensor, 12, [[144, 128], [24, 6], [1, 12]]),
                )
                if _dbg1 and q == 0 and h == 0:
                    nc.sync.dma_start(out=AP(dbg["dbg_py"], 0, [[72, 128], [1, 72]]), in_=py[:, :])
                # consolidate to out1p_d [32ch, (s, 12, 12)] in DRAM
                nc.sync.dma_start(
                    out=AP(
                        out1p_d.tensor,
                        (q * 4) * 144 + h * 72,
                        [[144, 4], [4608, 32], [1, 72]],
                    ),
                    in_=py[:, :],
                )
        # ---- duplicate out1p to 4 row-group bases ----
        dup4 = dup4pool.tile([128, 32 * 144], BF16)
        for r in range(4):
            nc.sync.dma_start(
                out=dup4[32 * r : 32 * r + 32, :],
                in_=AP(
                    out1p_d.tensor,
                    0,
                    [[4608, 32], [1, 4608]],
                ),
            )
        if _dbg1:
            nc.sync.dma_start(out=AP(dbg["dbg_dup4"], 0, [[4608, 128], [1, 4608]]), in_=dup4[:, :])
        # ---- conv2: 25 taps as K=32 row-group tiles ----
        o2s = []
        for cc in range(4):  # 8-sample chunks, N=512
            psA = pc2.tile([128, 512], F32, tag="ps2a")
            psB = pc2.tile([128, 512], F32, tag="ps2b")
            for t in range(25):
                r = t % 4
                slot = t // 4
                dy, dx = t // 5, t % 5
                ps = psA if r < 2 else psB
                colb = 64 * (r % 2)
                rhs = AP(
                    dup4.tensor,
                    32 * r * 4608 + cc * 8 * 144 + dy * 12 + dx,
                    [[4608, 32], [144, 8], [12, 8], [1, 8]],
                )
                nc.tensor.matmul(
                    ps[colb : colb + 64, :],
                    w2_sb[32 * r : 32 * r + 32, slot * 64 : slot * 64 + 64],
                    rhs,
                    start=(t < 4),
                    stop=(t >= 21),
                    tile_position=(32 * r, colb),
                )
            t1 = t2pool.tile([64, 512], F32, tag="t1")
            nc.vector.tensor_scalar_add(t1[:, :], psA[0:64, :], b2_sb[:, 0:1])
            t2 = t2pool.tile([64, 512], F32, tag="t2")
            nc.vector.tensor_add(t2[:, :], t1[:, :], psA[64:128, :])
            t1b = t2pool.tile([64, 512], F32, tag="t1")
            nc.vector.tensor_add(t1b[:, :], t2[:, :], psB[0:64, :])
            t3 = t2pool.tile([64, 512], F32, tag="t3")
            nc.vector.tensor_add(t3[:, :], t1b[:, :], psB[64:128, :])
            if _dbg1 and cc == 0:
                nc.sync.dma_start(out=AP(dbg["dbg_t3"], 0, [[512, 64], [1, 512]]), in_=t3[:, :])
            # maxpool 2x2 (f32, pre-relu: relu commutes w/ max)
            qx = p1pool.tile([64, 256], F32, tag="qx")
            nc.vector.tensor_max(
                qx[:, :],
                AP(t3.tensor, 0, [[512, 64], [64, 8], [8, 8], [2, 4]]),
                AP(t3.tensor, 1, [[512, 64], [64, 8], [8, 8], [2, 4]]),
            )
            qy = p1pool.tile([64, 128], F32, tag="qy")
            nc.vector.tensor_max(
                qy[:, :],
                AP(qx.tensor, 0, [[256, 64], [32, 8], [8, 4], [1, 4]]),
                AP(qx.tensor, 4, [[256, 64], [32, 8], [8, 4], [1, 4]]),
            )
            o2 = o2pool.tile([64, 128], BF16)
            nc.scalar.activation(o2[:, :], qy[:, :], ACTF.Relu)
            if _dbg1 and cc == 0:
                nc.sync.dma_start(out=AP(dbg["dbg_o2"], 0, [[128, 64], [1, 128]]), in_=o2[:, :])
            o2s.append(o2)
        # ---- build l3 [128=(2dup,64ci), (32s,16)] ----
        l3 = l3pool.tile([128, 512], BF16)
        for cc in range(4):
            for r in range(2):
                nc.sync.dma_start(
                    out=l3[64 * r : 64 * r + 64, cc * 128 : cc * 128 + 128],
                    in_=o2s[cc][:, :],
                )
        # ---- conv3: 9 taps, 2 row tiles (K=64), N=128 ----
        ps3 = pc1.tile([64, 128], F32, tag="ps1")
        for t in range(9):
            r = t % 2
            slot = t // 2
            dy, dx = t // 3, t % 3
            rhs = AP(
                l3.tensor,
                64 * r * 512 + dy * 4 + dx,
                [[512, 64], [16, 32], [4, 2], [1, 2]],
            )
            nc.tensor.matmul(
                ps3[32 * r : 32 * r + 10, :],
                w3_sb[64 * r : 64 * r + 64, slot * 10 : slot * 10 + 10],
                rhs,
                start=(t < 2),
                stop=(t >= 7),
                tile_position=(64 * r, 32 * r),
            )
        # fold partials + bias, relu, GAP
        lg0 = smpool.tile([16, 128], F32, tag="lg0")
        nc.vector.tensor_scalar_add(lg0[0:10, :], ps3[0:10, :], b3_sb[0:10, 0:1])
        lg = smpool.tile([16, 128], F32, tag="lg")
        nc.vector.tensor_add(lg[0:10, :], lg0[0:10, :], ps3[32:42, :])
        lgr = smpool.tile([16, 128], F32, tag="lgr")
        nc.vector.tensor_scalar(
            lgr[0:10, :], lg[0:10, :], 0.0, 0.25, ALU.max, ALU.mult
        )
        if _dbg1:
            nc.sync.dma_start(out=AP(dbg["dbg_lgr"], 0, [[128, 16], [1, 128]]), in_=lgr[:, :])
        nc.vector.tensor_reduce(
            out=logitsb[0:10, sub * 32 : sub * 32 + 32],
            in_=AP(lgr.tensor, 0, [[128, 10], [4, 32], [1, 4]]),
            axis=AXIS.X,
            op=ALU.add,
        )

    # ---- transpose [10, 128] -> [128, 10], log_softmax, store ----
    psT = pc1.tile([128, 16], F32, tag="ps1")
    nc.tensor.transpose(
        psT[:, 0:10], logitsb[0:10, :], ident10[0:10, 0:10]
    )
    if DEBUG_TAPS and bt == 0:
        nc.sync.dma_start(out=AP(dbg["dbg_logitsb"], 0, [[128, 16], [1, 128]]), in_=logitsb[:, :])
    mx = smpool.tile([128, 1], F32, tag="mx")
    nc.vector.reduce_max(mx[:, :], psT[:, 0:10], axis=AXIS.X)
    hs = smpool.tile([128, 16], F32, tag="hs")
    nc.vector.tensor_scalar(
        hs[:, 0:10], psT[:, 0:10], mx[:, 0:1], None, ALU.subtract
    )
    ex = smpool.tile([128, 16], F32, tag="ex")
    nc.scalar.activation(ex[:, 0:10], hs[:, 0:10], ACTF.Exp)
    sm = smpool.tile([128, 1], F32, tag="sm")
    nc.vector.reduce_sum(sm[:, :], ex[:, 0:10], axis=AXIS.X)
    lsm = smpool.tile([128, 1], F32, tag="lsm")
    nc.scalar.activation(lsm[:, :], sm[:, :], ACTF.Ln)
    outt = smpool.tile([128, 16], F32, tag="outt")
    nc.vector.tensor_scalar(
        outt[:, 0:10], hs[:, 0:10], lsm[:, 0:1], None, ALU.subtract
    )
    nc.sync.dma_start(
        out=AP(out_d, bt * 1280, [[10, 128], [1, 10]]),
        in_=outt[:, 0:10],
    )


